# revision 12
# baseline (speedup 1.0000x reference)
"""Trainium2 Bass kernel for the attention module:

    xp      = x @ W.T + b                      # [B, E]
    scores  = einsum('be,tbe->bt', xp, enc)    # [B, T]
    attn    = softmax(scores, axis=1)
    context = einsum('bt,tbe->be', attn, enc)  # [B, E]
    out     = concat([xp, context], axis=1)    # [B, 2E]

Shapes: T=2048, B=128, D_dec=512, E=512 (fp32).

Strategy (data-parallel over batch, 8 NeuronCores, no collectives):
  - Each core owns NB=16 batches: its encoder_states shard is
    [T, 16, E] = 64 MiB fp32, streamed from HBM exactly once in NT=16
    t-tiles of [128, 16, 512], CAST TO FP16 during the SWDGE DMA
    (HBM read stays fp32 -> memory roofline ~188us; SBUF side halves).
    The first NBUF DMAs are issued before anything else so the HBM
    stream starts at t=0; 5 rotating buffers give the stream ~3 tiles
    of lookahead.
  - fp16 on-chip enc buys: DVE 2x_1p mode for the big multiply, and
    1 cycle/row PE matmuls for the context accumulation (fp32 would be
    4 cycles/row = 2 passes).  Score accumulation stays fp32.
  - Per tile k (flash-style, deferred softmax combine):
      prod     = enc * xp            (ONE fp16 DVE tensor_tensor)
      S[t,b]   = sum_e prod          (batches 0-7: one batched DVE
                 tensor_reduce; 8-15: ScalarE activation accum)
      sT       = S^T                 (TensorE transpose, fp32)
      -m_k     = -rowmax(sT)         (VectorE reduce)
      pT, l_k  = exp(sT - m_k) + rowsum (ScalarE activation, pT fp16)
      p        = pT^T                (TensorE transpose, fp16)
      c_k      = per-batch sum_t p * enc via 16 masked fp16 matmuls
                 accumulating one [16, 512] fp32 PSUM tile (TensorE)
    Tails run TWO tiles behind heads (pipeline depth 3) so the
    cross-engine tail chain never throttles the DMA stream.
  - Final: exact softmax combine over the 16 tiles' (m_k, l_k, c_k),
    fp32, split between VectorE (tiles 0-7) and GpSimd (tiles 8-15)
    so the post-stream serial chain is short.

This toolchain's walrus accepts AT MOST ONE semaphore wait per TPB
compute instruction, and Tile pool slot reuse emits extra release
waits.  Hence: hot buffers are allocated once and alternated manually,
and cheap "observer" ops make each engine see a new producer before
the real consumer runs, keeping every instruction at <= 1 wait.
"""

import os
import sys

import numpy as np

if "/opt/trn_rl_repo" not in sys.path and not any(
    os.path.isdir(os.path.join(p, "concourse")) for p in sys.path if p
):
    sys.path.insert(0, "/opt/trn_rl_repo")

import concourse.bass as bass
import concourse.mybir as mybir
import concourse.tile as tile
from concourse.bass_utils import run_bass_kernel_spmd
from concourse.masks import make_identity
from concourse.tile_rust import add_dep_helper

T, B, D, E = 2048, 128, 512, 512
NCORES = 8
NB = B // NCORES  # 16 local batches per core
PT = 128          # t-tile partition size
NT = T // PT      # 16 t-tiles
NC_D = D // 128   # 4 chunks of the contraction dim for the xp matmul
NBUF = 5          # rotating fp16 enc tile buffers
NDVE = 8          # batches reduced by the batched DVE tensor_reduce
LAG = 2           # tail(k-LAG) is emitted after head(k)

F32 = mybir.dt.float32
F16 = mybir.dt.float16
AF = mybir.ActivationFunctionType
ALU = mybir.AluOpType
AX = mybir.AxisListType


def _install_drain_split():
    """This walrus rejects instructions carrying more than one semaphore
    wait.  Tile's kernel-tail drain waits on every proc's final tick in a
    single instruction; split it into one drain per wait."""
    from concourse.vector_clock import ScopedClock

    if getattr(tile.TileContext, "_drain_split_installed", False):
        return

    def _split_dab(self, tick_clock, wait_clock):
        drain_inst = self.nc.sync.drain()
        wait_clock.add_sem_waits(
            drain_inst.ins, ScopedClock({None: tick_clock.global_clock})
        )
        si = drain_inst.ins.sync_info
        if si is not None and len(si.on_wait) > 1:
            waits = list(si.on_wait)
            upds = list(si.on_update)
            drain_inst.ins.sync_info = mybir.SyncInfo(
                on_wait=[waits[0]], on_update=upds
            )
            for w in waits[1:]:
                d2 = self.nc.sync.drain()
                d2.ins.sync_info = mybir.SyncInfo(on_wait=[w], on_update=[])

        self.nc.all_engine_barrier()
        assert self.sems is not None
        popped = self.nc._tile_sem_poison_stack.pop()
        assert popped is self._sem_poison
        self.nc.clear_and_free_semaphores(list(self.sems.allocated().values()))
        self.nc.all_engine_barrier()

    tile.TileContext._drain_and_barrier = _split_dab
    tile.TileContext._drain_split_installed = True


_install_drain_split()


def build_nc() -> bass.Bass:
    nc = bass.Bass()

    # Per-core shards (host pre-transposes the small operands for layout).
    xT_ext = nc.declare_dram_parameter("xT", [D, NB], F32, isOutput=False)
    WT_ext = nc.declare_dram_parameter("WT", [D, E], F32, isOutput=False)
    b_ext = nc.declare_dram_parameter("bias", [128, NC_D], F32, isOutput=False)
    enc_ext = nc.declare_dram_parameter("enc", [T, NB, E], F32, isOutput=False)
    out_ext = nc.declare_dram_parameter("out", [NB, 2 * E], F32, isOutput=True)

    with tile.TileContext(nc) as tc:
        with (
            tc.tile_pool(name="sb", bufs=1) as sb,
            tc.tile_pool(name="dram", bufs=1, space="DRAM") as dram_pool,
            tc.tile_pool(name="ps", bufs=1, space="PSUM") as ps,
        ):
            # ---- enc stream starts immediately ---------------------------
            # The first NBUF tile DMAs have no hazards; issuing them before
            # everything else puts the 64 MiB HBM stream in flight while the
            # xp setup chain runs.
            enc_t = [sb.tile([PT, NB, E], F16, name=f"enc{i}") for i in range(NBUF)]
            dmas = {}
            for k in range(NBUF):
                dmas[k] = nc.gpsimd.dma_start(
                    out=enc_t[k][:], in_=enc_ext[k * PT : (k + 1) * PT, :, :]
                )

            ident = sb.tile([128, 128], F32)
            make_identity(nc, ident[:])
            identH = sb.tile([128, 128], F16)
            make_identity(nc, identH[:])

            obs_ps1 = ps.tile([1, 16], F32, tag="obs1")
            obs_ps2 = ps.tile([1, 16], F32, tag="obs2")

            def pe_observe(ap, obs):
                return nc.tensor.matmul(obs[:], lhsT=ap[:, 0:1], rhs=ap[:, 0:16],
                                        start=True, stop=True)

            pe_observe(ident, obs_ps1)   # PE observes the fp32 identity
            pe_observe(identH, obs_ps1)  # PE observes the fp16 identity

            # ---- setup: xp = x @ W.T + b ----------------------------------
            xT_sb = sb.tile([128, NC_D, NB], F32)
            nc.sync.dma_start(
                out=xT_sb[:], in_=xT_ext[:, :].rearrange("(c p) b -> p c b", p=128)
            )
            WT_sb = sb.tile([128, NC_D, E], F32)
            nc.sync.dma_start(
                out=WT_sb[:], in_=WT_ext[:, :].rearrange("(c p) e -> p c e", p=128)
            )
            b_sb = sb.tile([128, NC_D], F32)
            nc.sync.dma_start(out=b_sb[:], in_=b_ext[:, :])

            obs_xt = pe_observe(xT_sb[:, 0, :], obs_ps2)  # PE observes xT DMA
            junk_b = sb.tile([128, 1], F32)
            nc.vector.tensor_copy(junk_b[:], b_sb[:, 0:1])  # DVE observes b DMA

            # xp^T chunks: [128 (e-local), ce, b]
            xpT_sb = sb.tile([128, NC_D, NB], F32)
            ps_xpT = ps.tile([128, NB], F32, tag="ps_xpT")
            for ce in range(NC_D):
                for cd in range(NC_D):
                    mm = nc.tensor.matmul(
                        ps_xpT[:],
                        lhsT=WT_sb[:, cd, ce * 128 : (ce + 1) * 128],
                        rhs=xT_sb[:, cd, :],
                        start=(cd == 0),
                        stop=(cd == NC_D - 1),
                    )
                    if ce == 0 and cd == 0:
                        add_dep_helper(mm.ins, obs_xt.ins, sync=False)
                nc.vector.tensor_scalar_add(
                    xpT_sb[:, ce, :], ps_xpT[:], b_sb[:, ce : ce + 1]
                )

            # out[:, 0:E] = xp (natural layout) via TensorE transposes
            out_tile = sb.tile([NB, 2 * E], F32)
            ps_xp = ps.tile([NB, 128], F32, tag="ps_xp")
            for ce in range(NC_D):
                nc.tensor.transpose(ps_xp[:], xpT_sb[:, ce, :], ident[:])
                # DVE (not ACT) so out_tile has a single producer engine
                nc.vector.tensor_copy(
                    out_tile[:, ce * 128 : (ce + 1) * 128], ps_xp[:]
                )

            # Broadcast xp (cast fp16) to all 128 partitions via DRAM bounce
            # with a 0-stride partition dim on the read side.  The cast
            # happens on the SWDGE write leg; the big broadcast read leg is
            # HWDGE fp16->fp16.
            xp_dram = dram_pool.tile([NB, E], F16)
            nc.gpsimd.dma_start(out=xp_dram[:], in_=out_tile[:, 0:E])
            xpb = sb.tile([128, NB, E], F16)
            nc.sync.dma_start(out=xpb[:], in_=xp_dram[:].partition_broadcast(128))

            # ---- persistent buffers for the t-tile loop -------------------
            # Write-once column layouts (one column/slice per t-tile) avoid
            # same-engine WAW hazards entirely; prod/pm/ctx alternate
            # manually.
            prod = [sb.tile([PT, NB, E], F16, name=f"prod{i}") for i in range(2)]
            S_all = sb.tile([PT, NT, NB], F32)
            pT_all = sb.tile([NB, NT, PT], F16)
            pm_t = [sb.tile([PT, NB, NB], F16, name=f"pm{i}") for i in range(2)]
            nc.vector.memset(pm_t[0][:], 0.0)  # off-diagonals stay 0 forever
            nc.vector.memset(pm_t[1][:], 0.0)
            jpm = sb.tile([1, NT], F32)
            junk_es = sb.tile([PT, NT], F32)
            junk_ss = sb.tile([NB, NT], F32)
            dummy_all = sb.tile([PT, NT, NB], F32)  # write-once reduce dummies
            jw = sb.tile([1, 1], F32)
            jns = sb.tile([NB, NT], F32)

            sT_ps = ps.tile([NB, PT], F32, tag="sT")
            p_ps = ps.tile([PT, NB], F16, tag="p")
            ctx_pair = [ps.tile([NB, E], F32, name=f"ctx{i}") for i in range(2)]

            NEGM = sb.tile([NB, NT], F32)   # -m_k per (b, k)
            L_all = sb.tile([NB, NT], F32)  # l_k per (b, k)
            c_store = sb.tile([NB, NT, E], F32)

            # ---- software-pipelined t-tile loop --------------------------
            # head(k): enc DMA + multiply + reduces for tile k
            # tail(k-2): transposes/softmax/context matmuls for tile k-2,
            # emitted after head(k) so the cross-engine tail chain (which is
            # longer than one DMA period) is pipelined three tiles deep.
            hist = {}
            tts = {}
            cstore_q = []  # tiles whose ctx_ps copy is still pending

            def emit_tail(k):
                eth = enc_t[k % NBUF]
                ctx_ps = ctx_pair[k % 2]
                st_inst = nc.tensor.transpose(sT_ps[:], S_all[:, k, :], ident[0:PT, 0:PT])
                if cstore_q:
                    # the pending c_store copy must land before this tile's
                    # ctx matmuls reuse ctx_ps; syncing the S transpose on it
                    # both orders the streams and keeps jmm at one wait
                    add_dep_helper(st_inst.ins, cstore_q[-1].ins, sync=True)
                if k >= 1:
                    # DVE observes exp(k-1) (the last sT_ps reader) so the
                    # NEGM reduce carries only the PE wait
                    nc.vector.tensor_copy(jns[:, k : k + 1], pT_all[:, k - 1, 0:1])
                nc.vector.tensor_reduce(
                    out=NEGM[:, k : k + 1], in_=sT_ps[:], axis=AX.X, op=ALU.max,
                    negate=True,
                )
                nc.scalar.activation(junk_ss[:, k : k + 1], sT_ps[:, 0:1], AF.Copy)
                nc.scalar.activation(
                    pT_all[:, k, :], sT_ps[:], AF.Exp,
                    bias=NEGM[:, k : k + 1], scale=1.0,
                    accum_out=L_all[:, k : k + 1],
                )
                nc.tensor.transpose(p_ps[:], pT_all[:, k, :], identH[0:NB, 0:NB])
                pm = pm_t[k % 2]
                if k >= 2:
                    # DVE observes its own k-2 diagonal write so the diag
                    # copy below carries only the PE wait
                    nc.vector.tensor_copy(jpm[:, k : k + 1], pm[0:1, 0, 0:1])
                nc.vector.tensor_copy(
                    pm[:, :, :].rearrange("p a b -> p (a b)")[:, :: NB + 1],
                    p_ps[:],
                )
                # PE observes the enc DMA before the ctx matmuls use it
                jmm = nc.tensor.matmul(
                    ctx_ps[0:1, 0:16],
                    lhsT=eth[:, 0, 0:1], rhs=eth[:, 0, 0:16],
                    start=True, stop=True,
                )
                add_dep_helper(jmm.ins, st_inst.ins, sync=False)
                last_mm = None
                for b in range(NB):
                    last_mm = nc.tensor.matmul(
                        ctx_ps[:],
                        lhsT=pm[:, b, :],
                        rhs=eth[:, b, :],
                        start=(b == 0),
                        stop=(b == NB - 1),
                    )
                hist[k % NBUF] = (last_mm, tts[k], dmas[k])
                return last_mm

            def emit_cstore(k):
                inst = nc.scalar.activation(
                    c_store[:, k, :], ctx_pair[k % 2][:], AF.Copy
                )
                cstore_q.append(inst)
                return inst

            for k in range(NT):
                eth = enc_t[k % NBUF]
                pr = prod[k % 2]
                # Pre-absorb the buffer's reuse hazards on the Pool proc
                # with explicitly-synced nops, so the SWDGE DMA needs no
                # more than the allowed number of waits.
                if k >= NBUF:
                    for dep in hist[k % NBUF]:
                        nop = nc.gpsimd.engine_nop()
                        add_dep_helper(nop.ins, dep.ins, sync=True)
                    dmas[k] = nc.gpsimd.dma_start(
                        out=eth[:], in_=enc_ext[k * PT : (k + 1) * PT, :, :]
                    )
                # DVE observes the DMA so the multiply's enc wait elides
                nc.vector.tensor_copy(junk_es[:, k : k + 1], eth[:, 0, 0:1])

                # prod = enc * xp for all 16 batches (one fp16 DVE op).
                # Its one cross-engine hazard (WAR vs tile k-2's ACT reduces
                # of prod) elides via the jns observer chain.
                tts[k] = nc.vector.tensor_tensor(
                    out=pr[:], in0=eth[:], in1=xpb[:], op=ALU.mult,
                )

                # Scores: batched DVE reduce for the first NDVE batches
                # (tile 0: all 16, so S_all[:,0,:] has a single producer
                # engine and the first tail's transpose carries one wait),
                # ScalarE activation-accumulate for the rest.
                nd = NB if k == 0 else NDVE
                nc.vector.tensor_reduce(
                    out=S_all[:, k, 0:nd], in_=pr[:, 0:nd, :], axis=AX.X,
                    op=ALU.add,
                )
                for i, b in enumerate(range(nd, NB)):
                    nc.scalar.activation(
                        dummy_all[:, k, b : b + 1].broadcast_to((PT, E)),
                        pr[:, b, :],
                        AF.Copy,
                        accum_out=S_all[:, k, b : b + 1],
                    )
                    if i == 0 and k >= LAG + 1:
                        emit_cstore(k - LAG - 1)

                if k >= LAG:
                    emit_tail(k - LAG)

            emit_tail(NT - 2)
            emit_cstore(NT - 3)
            emit_tail(NT - 1)
            emit_cstore(NT - 2)
            emit_cstore(NT - 1)
            prev_cstore = cstore_q[-1]

            # ---- final combine across tiles -------------------------------
            negM = sb.tile([NB, 1], F32)
            nc.vector.tensor_reduce(out=negM[:], in_=NEGM[:], axis=AX.X, op=ALU.min)
            alpha = sb.tile([NB, NT], F32)
            # alpha = exp(-NEGM * 1 + (-M)) = exp(m_k - M); ordered after the
            # last c_store copy so the combine loop's ACT waits all elide
            ainst = nc.scalar.activation(
                alpha[:], NEGM[:], AF.Exp, bias=negM[:], scale=-1.0
            )
            add_dep_helper(ainst.ins, prev_cstore.ins, sync=False)
            prodw = sb.tile([NB, NT], F32)
            nc.vector.tensor_tensor(out=prodw[:], in0=alpha[:], in1=L_all[:],
                                    op=ALU.mult)
            Lsum = sb.tile([NB, 1], F32)
            nc.vector.tensor_reduce(out=Lsum[:], in_=prodw[:], axis=AX.X, op=ALU.add)
            rL = sb.tile([NB, 1], F32)
            nc.vector.reciprocal(rL[:], Lsum[:])
            w = sb.tile([NB, NT], F32)
            nc.vector.tensor_scalar_mul(w[:], alpha[:], rL[:])

            # Weighted sum of the 16 stored contexts on DVE as TWO
            # interleaved accumulation chains, so each op's posted-write
            # self-wait hides under the other chain's execution.  All ACT
            # (c_store) waits elide because prodw already waited on alpha,
            # which is ordered after the last c_store copy.
            acc = out_tile[:, E : 2 * E]
            acc_b = sb.tile([NB, E], F32)
            nc.vector.tensor_copy(jw[:], w[0:1, 0:1])  # absorb w's self-wait
            nc.vector.tensor_scalar_mul(acc, c_store[:, 0, :], w[:, 0:1])
            h = NT // 2
            nc.vector.tensor_scalar_mul(acc_b[:], c_store[:, h, :], w[:, h : h + 1])
            for k in range(1, h):
                nc.vector.scalar_tensor_tensor(
                    out=acc, in0=c_store[:, k, :], scalar=w[:, k : k + 1], in1=acc,
                    op0=ALU.mult, op1=ALU.add,
                )
                nc.vector.scalar_tensor_tensor(
                    out=acc_b[:], in0=c_store[:, h + k, :],
                    scalar=w[:, h + k : h + k + 1], in1=acc_b[:],
                    op0=ALU.mult, op1=ALU.add,
                )
            nc.vector.tensor_tensor(out=acc, in0=acc, in1=acc_b[:], op=ALU.add)

            nc.sync.dma_start(out=out_ext[:, :], in_=out_tile[:])

    return nc


_NC_CACHE: bass.Bass | None = None


def _get_nc() -> bass.Bass:
    global _NC_CACHE
    if _NC_CACHE is None:
        _NC_CACHE = build_nc()
    return _NC_CACHE


def make_in_maps(inputs: dict) -> list[dict]:
    x = np.ascontiguousarray(np.asarray(inputs["x"], dtype=np.float32))
    enc = np.asarray(inputs["encoder_states"], dtype=np.float32)
    W = np.asarray(inputs["W"], dtype=np.float32)
    bias = np.asarray(inputs["b"], dtype=np.float32)

    WT = np.ascontiguousarray(W.T)
    b128 = np.ascontiguousarray(bias.reshape(NC_D, 128).T)
    in_maps = []
    for i in range(NCORES):
        sl = slice(i * NB, (i + 1) * NB)
        in_maps.append(
            {
                "xT": np.ascontiguousarray(x[sl].T),
                "WT": WT,
                "bias": b128,
                "enc": np.ascontiguousarray(enc[:, sl, :]),
            }
        )
    return in_maps


def run(inputs: dict, trace: bool = False, tmpdir: str | None = None):
    """Returns (full_output [B, 2E] f32, exec_time_ns or None)."""
    nc = _get_nc()
    in_maps = make_in_maps(inputs)
    res = run_bass_kernel_spmd(
        nc, in_maps, core_ids=list(range(NCORES)), trace=trace, tmpdir=tmpdir
    )
    out = np.concatenate([res.results[i]["out"] for i in range(NCORES)], axis=0)
    return out.astype(np.float32), res.exec_time_ns


def kernel(**inputs) -> np.ndarray:
    out, _ = run(inputs, trace=False)
    return out


# revision 20
# speedup vs baseline: 1.0350x; 1.0350x over previous
"""Trainium2 Bass kernel for the attention module:

    xp      = x @ W.T + b                      # [B, E]
    scores  = einsum('be,tbe->bt', xp, enc)    # [B, T]
    attn    = softmax(scores, axis=1)
    context = einsum('bt,tbe->be', attn, enc)  # [B, E]
    out     = concat([xp, context], axis=1)    # [B, 2E]

Shapes: T=2048, B=128, D_dec=512, E=512 (fp32).

Strategy (data-parallel over batch, 8 NeuronCores, no collectives):
  - Each core owns NB=16 batches: its encoder_states shard is
    [T, 16, E] = 64 MiB fp32, streamed from HBM exactly once in NT=16
    t-tiles of [128, 16, 512], CAST TO FP16 during the SWDGE DMA
    (HBM read stays fp32 -> memory roofline ~188us; SBUF side halves).
    The first NBUF DMAs are issued before anything else so the HBM
    stream starts at t=0; 5 rotating buffers give the stream ~3 tiles
    of lookahead.
  - fp16 on-chip enc buys: DVE 2x_1p mode for the big multiply, and
    1 cycle/row PE matmuls for the context accumulation (fp32 would be
    4 cycles/row = 2 passes).  Score accumulation stays fp32.
  - Per tile k (flash-style, deferred softmax combine):
      prod     = enc * xp            (ONE fp16 DVE tensor_tensor)
      S[t,b]   = sum_e prod          (batches 0-7: one batched DVE
                 tensor_reduce; 8-15: ScalarE activation accum)
      sT       = S^T                 (TensorE transpose, fp32)
      -m_k     = -rowmax(sT)         (VectorE reduce)
      pT, l_k  = exp(sT - m_k) + rowsum (ScalarE activation, pT fp16)
      p        = pT^T                (TensorE transpose, fp16)
      c_k      = per-batch sum_t p * enc via 16 masked fp16 matmuls
                 accumulating one [16, 512] fp32 PSUM tile (TensorE)
    Tails run TWO tiles behind heads (pipeline depth 3) so the
    cross-engine tail chain never throttles the DMA stream.
  - Final: exact softmax combine over the 16 tiles' (m_k, l_k, c_k),
    fp32, split between VectorE (tiles 0-7) and GpSimd (tiles 8-15)
    so the post-stream serial chain is short.

This toolchain's walrus accepts AT MOST ONE semaphore wait per TPB
compute instruction, and Tile pool slot reuse emits extra release
waits.  Hence: hot buffers are allocated once and alternated manually,
and cheap "observer" ops make each engine see a new producer before
the real consumer runs, keeping every instruction at <= 1 wait.
"""

import os
import sys

import numpy as np

if "/opt/trn_rl_repo" not in sys.path and not any(
    os.path.isdir(os.path.join(p, "concourse")) for p in sys.path if p
):
    sys.path.insert(0, "/opt/trn_rl_repo")

import concourse.bass as bass
import concourse.mybir as mybir
import concourse.tile as tile
from concourse.bass_utils import run_bass_kernel_spmd
from concourse.masks import make_identity
from concourse.tile_rust import add_dep_helper

T, B, D, E = 2048, 128, 512, 512
NCORES = 8
NB = B // NCORES  # 16 local batches per core
PT = 128          # t-tile partition size
NT = T // PT      # 16 t-tiles
NC_D = D // 128   # 4 chunks of the contraction dim for the xp matmul
NBUF = 6          # rotating fp16 enc tile buffers
NDVE = 6          # batches reduced by the batched DVE tensor_reduce
LAG = 2           # tail(k-LAG) is emitted after head(k)

F32 = mybir.dt.float32
F16 = mybir.dt.float16
AF = mybir.ActivationFunctionType
ALU = mybir.AluOpType
AX = mybir.AxisListType


def _install_drain_split():
    """This walrus rejects instructions carrying more than one semaphore
    wait.  Tile's kernel-tail drain waits on every proc's final tick in a
    single instruction; split it into one drain per wait."""
    from concourse.vector_clock import ScopedClock

    if getattr(tile.TileContext, "_drain_split_installed", False):
        return

    def _split_dab(self, tick_clock, wait_clock):
        drain_inst = self.nc.sync.drain()
        wait_clock.add_sem_waits(
            drain_inst.ins, ScopedClock({None: tick_clock.global_clock})
        )
        si = drain_inst.ins.sync_info
        if si is not None and len(si.on_wait) > 1:
            waits = list(si.on_wait)
            upds = list(si.on_update)
            drain_inst.ins.sync_info = mybir.SyncInfo(
                on_wait=[waits[0]], on_update=upds
            )
            for w in waits[1:]:
                d2 = self.nc.sync.drain()
                d2.ins.sync_info = mybir.SyncInfo(on_wait=[w], on_update=[])

        self.nc.all_engine_barrier()
        assert self.sems is not None
        popped = self.nc._tile_sem_poison_stack.pop()
        assert popped is self._sem_poison
        self.nc.clear_and_free_semaphores(list(self.sems.allocated().values()))
        self.nc.all_engine_barrier()

    tile.TileContext._drain_and_barrier = _split_dab
    tile.TileContext._drain_split_installed = True


_install_drain_split()


def build_nc() -> bass.Bass:
    nc = bass.Bass()

    # Per-core shards (host pre-transposes the small operands for layout).
    xT_ext = nc.declare_dram_parameter("xT", [D, NB], F32, isOutput=False)
    WT_ext = nc.declare_dram_parameter("WT", [D, E], F32, isOutput=False)
    b_ext = nc.declare_dram_parameter("bias", [128, NC_D], F32, isOutput=False)
    enc_ext = nc.declare_dram_parameter("enc", [T, NB, E], F32, isOutput=False)
    out_ext = nc.declare_dram_parameter("out", [NB, 2 * E], F32, isOutput=True)

    with tile.TileContext(nc) as tc:
        with (
            tc.tile_pool(name="sb", bufs=1) as sb,
            tc.tile_pool(name="dram", bufs=1, space="DRAM") as dram_pool,
            tc.tile_pool(name="ps", bufs=1, space="PSUM") as ps,
        ):
            # ---- enc stream starts immediately ---------------------------
            # The first NBUF tile DMAs have no hazards; issuing them before
            # everything else puts the 64 MiB HBM stream in flight while the
            # xp setup chain runs.
            enc_t = [sb.tile([PT, NB, E], F16, name=f"enc{i}") for i in range(NBUF)]
            dmas = {}
            for k in range(NBUF):
                dmas[k] = nc.gpsimd.dma_start(
                    out=enc_t[k][:], in_=enc_ext[k * PT : (k + 1) * PT, :, :]
                )

            ident = sb.tile([128, 128], F32)
            make_identity(nc, ident[:])
            identH = sb.tile([128, 128], F16)
            make_identity(nc, identH[:])

            obs_ps1 = ps.tile([1, 16], F32, tag="obs1")
            obs_ps2 = ps.tile([1, 16], F32, tag="obs2")

            def pe_observe(ap, obs):
                return nc.tensor.matmul(obs[:], lhsT=ap[:, 0:1], rhs=ap[:, 0:16],
                                        start=True, stop=True)

            pe_observe(ident, obs_ps1)   # PE observes the fp32 identity
            pe_observe(identH, obs_ps1)  # PE observes the fp16 identity

            # ---- setup: xp = x @ W.T + b ----------------------------------
            xT_sb = sb.tile([128, NC_D, NB], F32)
            nc.sync.dma_start(
                out=xT_sb[:], in_=xT_ext[:, :].rearrange("(c p) b -> p c b", p=128)
            )
            WT_sb = sb.tile([128, NC_D, E], F32)
            nc.sync.dma_start(
                out=WT_sb[:], in_=WT_ext[:, :].rearrange("(c p) e -> p c e", p=128)
            )
            b_sb = sb.tile([128, NC_D], F32)
            nc.sync.dma_start(out=b_sb[:], in_=b_ext[:, :])

            obs_xt = pe_observe(xT_sb[:, 0, :], obs_ps2)  # PE observes xT DMA
            junk_b = sb.tile([128, 1], F32)
            nc.vector.tensor_copy(junk_b[:], b_sb[:, 0:1])  # DVE observes b DMA

            # xp^T chunks: [128 (e-local), ce, b]
            xpT_sb = sb.tile([128, NC_D, NB], F32)
            ps_xpT = ps.tile([128, NB], F32, tag="ps_xpT")
            for ce in range(NC_D):
                for cd in range(NC_D):
                    mm = nc.tensor.matmul(
                        ps_xpT[:],
                        lhsT=WT_sb[:, cd, ce * 128 : (ce + 1) * 128],
                        rhs=xT_sb[:, cd, :],
                        start=(cd == 0),
                        stop=(cd == NC_D - 1),
                    )
                    if ce == 0 and cd == 0:
                        add_dep_helper(mm.ins, obs_xt.ins, sync=False)
                nc.vector.tensor_scalar_add(
                    xpT_sb[:, ce, :], ps_xpT[:], b_sb[:, ce : ce + 1]
                )

            # out[:, 0:E] = xp (natural layout) via TensorE transposes
            out_tile = sb.tile([NB, 2 * E], F32)
            ps_xp = ps.tile([NB, 128], F32, tag="ps_xp")
            for ce in range(NC_D):
                nc.tensor.transpose(ps_xp[:], xpT_sb[:, ce, :], ident[:])
                # DVE (not ACT) so out_tile has a single producer engine
                nc.vector.tensor_copy(
                    out_tile[:, ce * 128 : (ce + 1) * 128], ps_xp[:]
                )

            # The xp half of the output never changes after this point; ship
            # it now so the final output DMA only covers the context half.
            nc.sync.dma_start(out=out_ext[:, 0:E], in_=out_tile[:, 0:E])

            # Broadcast xp (as fp16) to all 128 partitions via DRAM bounce
            # with a 0-stride partition dim on the read side.  Cast on-chip
            # first so BOTH DMA legs ride the HWDGE queue — the Pool/SWDGE
            # queue is busy streaming enc tiles and would serialize this
            # behind them.
            xp16 = sb.tile([NB, E], F16)
            nc.vector.tensor_copy(xp16[:], out_tile[:, 0:E])
            xp_dram = dram_pool.tile([NB, E], F16)
            nc.sync.dma_start(out=xp_dram[:], in_=xp16[:])
            xpb = sb.tile([128, NB, E], F16)
            nc.sync.dma_start(out=xpb[:], in_=xp_dram[:].partition_broadcast(128))

            # ---- persistent buffers for the t-tile loop -------------------
            # Write-once column layouts (one column/slice per t-tile) avoid
            # same-engine WAW hazards entirely; prod/pm/ctx alternate
            # manually.
            prod = [sb.tile([PT, NB, E], F16, name=f"prod{i}") for i in range(2)]
            S_all = sb.tile([PT, NT, NB], F32)
            pT_all = sb.tile([NB, NT, PT], F16)
            pm_t = [sb.tile([PT, NB, NB], F16, name=f"pm{i}") for i in range(2)]
            nc.vector.memset(pm_t[0][:], 0.0)  # off-diagonals stay 0 forever
            nc.vector.memset(pm_t[1][:], 0.0)
            jpm = sb.tile([1, NT], F32)
            junk_es = sb.tile([PT, NT], F32)
            junk_ss = sb.tile([NB, NT], F32)
            dummy_all = sb.tile([PT, NT, NB], F32)  # write-once reduce dummies
            jw = sb.tile([1, 1], F32)
            jns = sb.tile([NB, NT], F32)

            sT_ps = ps.tile([NB, PT], F32, tag="sT")
            p_ps = ps.tile([PT, NB], F16, tag="p")
            ctx_pair = [ps.tile([NB, E], F32, name=f"ctx{i}") for i in range(2)]

            NEGM = sb.tile([NB, NT], F32)   # -m_k per (b, k)
            L_all = sb.tile([NB, NT], F32)  # l_k per (b, k)
            c_store = sb.tile([NB, NT, E], F16)

            # ---- software-pipelined t-tile loop --------------------------
            # head(k): enc DMA + multiply + reduces for tile k
            # tail(k-2): transposes/softmax/context matmuls for tile k-2,
            # emitted after head(k) so the cross-engine tail chain (which is
            # longer than one DMA period) is pipelined three tiles deep.
            hist = {}
            tts = {}
            cstore_q = []  # tiles whose ctx_ps copy is still pending

            def emit_tail(k):
                eth = enc_t[k % NBUF]
                ctx_ps = ctx_pair[k % 2]
                st_inst = nc.tensor.transpose(sT_ps[:], S_all[:, k, :], ident[0:PT, 0:PT])
                if cstore_q:
                    # the pending c_store copy must land before this tile's
                    # ctx matmuls reuse ctx_ps; syncing the S transpose on it
                    # both orders the streams and keeps jmm at one wait
                    add_dep_helper(st_inst.ins, cstore_q[-1].ins, sync=True)
                if k >= 1:
                    # DVE observes exp(k-1) (the last sT_ps reader) so the
                    # NEGM reduce carries only the PE wait
                    nc.vector.tensor_copy(jns[:, k : k + 1], pT_all[:, k - 1, 0:1])
                nc.vector.tensor_reduce(
                    out=NEGM[:, k : k + 1], in_=sT_ps[:], axis=AX.X, op=ALU.max,
                    negate=True,
                )
                nc.scalar.activation(junk_ss[:, k : k + 1], sT_ps[:, 0:1], AF.Copy)
                nc.scalar.activation(
                    pT_all[:, k, :], sT_ps[:], AF.Exp,
                    bias=NEGM[:, k : k + 1], scale=1.0,
                    accum_out=L_all[:, k : k + 1],
                )
                nc.tensor.transpose(p_ps[:], pT_all[:, k, :], identH[0:NB, 0:NB])
                pm = pm_t[k % 2]
                if k >= 2:
                    # DVE observes its own k-2 diagonal write so the diag
                    # copy below carries only the PE wait
                    nc.vector.tensor_copy(jpm[:, k : k + 1], pm[0:1, 0, 0:1])
                nc.vector.tensor_copy(
                    pm[:, :, :].rearrange("p a b -> p (a b)")[:, :: NB + 1],
                    p_ps[:],
                )
                # PE observes the enc DMA before the ctx matmuls use it
                jmm = nc.tensor.matmul(
                    ctx_ps[0:1, 0:16],
                    lhsT=eth[:, 0, 0:1], rhs=eth[:, 0, 0:16],
                    start=True, stop=True,
                )
                add_dep_helper(jmm.ins, st_inst.ins, sync=False)
                last_mm = None
                for b in range(NB):
                    last_mm = nc.tensor.matmul(
                        ctx_ps[:],
                        lhsT=pm[:, b, :],
                        rhs=eth[:, b, :],
                        start=(b == 0),
                        stop=(b == NB - 1),
                    )
                hist[k % NBUF] = (last_mm, tts[k], dmas[k])
                return last_mm

            def emit_cstore(k):
                inst = nc.scalar.activation(
                    c_store[:, k, :], ctx_pair[k % 2][:], AF.Copy
                )
                cstore_q.append(inst)
                return inst

            for k in range(NT):
                eth = enc_t[k % NBUF]
                pr = prod[k % 2]
                # Pre-absorb the buffer's reuse hazards on the Pool proc
                # with explicitly-synced nops, so the SWDGE DMA needs no
                # more than the allowed number of waits.
                if k >= NBUF:
                    for dep in hist[k % NBUF]:
                        nop = nc.gpsimd.engine_nop()
                        add_dep_helper(nop.ins, dep.ins, sync=True)
                    dmas[k] = nc.gpsimd.dma_start(
                        out=eth[:], in_=enc_ext[k * PT : (k + 1) * PT, :, :]
                    )
                # DVE observes the DMA so the multiply's enc wait elides
                nc.vector.tensor_copy(junk_es[:, k : k + 1], eth[:, 0, 0:1])

                # prod = enc * xp for all 16 batches (one fp16 DVE op).
                # Its one cross-engine hazard (WAR vs tile k-2's ACT reduces
                # of prod) elides via the jns observer chain.
                tts[k] = nc.vector.tensor_tensor(
                    out=pr[:], in0=eth[:], in1=xpb[:], op=ALU.mult,
                )

                # Scores: batched DVE reduce for the first NDVE batches
                # (tile 0: all 16, so S_all[:,0,:] has a single producer
                # engine and the first tail's transpose carries one wait),
                # ScalarE activation-accumulate for the rest.
                nd = NB if k == 0 else NDVE
                nc.vector.tensor_reduce(
                    out=S_all[:, k, 0:nd], in_=pr[:, 0:nd, :], axis=AX.X,
                    op=ALU.add,
                )
                for i, b in enumerate(range(nd, NB)):
                    nc.scalar.activation(
                        dummy_all[:, k, b : b + 1].broadcast_to((PT, E)),
                        pr[:, b, :],
                        AF.Copy,
                        accum_out=S_all[:, k, b : b + 1],
                    )
                    if i == 0 and k >= LAG + 1:
                        emit_cstore(k - LAG - 1)

                if k >= LAG:
                    emit_tail(k - LAG)

            emit_tail(NT - 2)
            emit_cstore(NT - 3)
            emit_tail(NT - 1)
            emit_cstore(NT - 2)
            emit_cstore(NT - 1)
            prev_cstore = cstore_q[-1]

            # ---- final combine across tiles -------------------------------
            negM = sb.tile([NB, 1], F32)
            nc.vector.tensor_reduce(out=negM[:], in_=NEGM[:], axis=AX.X, op=ALU.min)
            alpha = sb.tile([NB, NT], F32)
            # alpha = exp(-NEGM * 1 + (-M)) = exp(m_k - M); ordered after the
            # last c_store copy so the combine loop's ACT waits all elide
            ainst = nc.scalar.activation(
                alpha[:], NEGM[:], AF.Exp, bias=negM[:], scale=-1.0
            )
            add_dep_helper(ainst.ins, prev_cstore.ins, sync=False)
            prodw = sb.tile([NB, NT], F32)
            nc.vector.tensor_tensor(out=prodw[:], in0=alpha[:], in1=L_all[:],
                                    op=ALU.mult)
            Lsum = sb.tile([NB, 1], F32)
            nc.vector.tensor_reduce(out=Lsum[:], in_=prodw[:], axis=AX.X, op=ALU.add)
            rL = sb.tile([NB, 1], F32)
            nc.vector.reciprocal(rL[:], Lsum[:])
            w = sb.tile([NB, NT], F32)
            nc.vector.tensor_scalar_mul(w[:], alpha[:], rL[:])

            # Weighted sum of the 16 stored contexts on DVE as TWO
            # interleaved accumulation chains, so each op's posted-write
            # self-wait hides under the other chain's execution.  All ACT
            # (c_store) waits elide because prodw already waited on alpha,
            # which is ordered after the last c_store copy.
            acc = out_tile[:, E : 2 * E]
            acc_b = sb.tile([NB, E], F32)
            nc.vector.tensor_copy(jw[:], w[0:1, 0:1])  # absorb w's self-wait
            nc.vector.tensor_scalar_mul(acc, c_store[:, 0, :], w[:, 0:1])
            h = NT // 2
            nc.vector.tensor_scalar_mul(acc_b[:], c_store[:, h, :], w[:, h : h + 1])
            for k in range(1, h):
                nc.vector.scalar_tensor_tensor(
                    out=acc, in0=c_store[:, k, :], scalar=w[:, k : k + 1], in1=acc,
                    op0=ALU.mult, op1=ALU.add,
                )
                nc.vector.scalar_tensor_tensor(
                    out=acc_b[:], in0=c_store[:, h + k, :],
                    scalar=w[:, h + k : h + k + 1], in1=acc_b[:],
                    op0=ALU.mult, op1=ALU.add,
                )
            nc.vector.tensor_tensor(out=acc, in0=acc, in1=acc_b[:], op=ALU.add)

            nc.sync.dma_start(out=out_ext[:, E : 2 * E], in_=acc)

    return nc


_NC_CACHE: bass.Bass | None = None


def _get_nc() -> bass.Bass:
    global _NC_CACHE
    if _NC_CACHE is None:
        _NC_CACHE = build_nc()
    return _NC_CACHE


def make_in_maps(inputs: dict) -> list[dict]:
    x = np.ascontiguousarray(np.asarray(inputs["x"], dtype=np.float32))
    enc = np.asarray(inputs["encoder_states"], dtype=np.float32)
    W = np.asarray(inputs["W"], dtype=np.float32)
    bias = np.asarray(inputs["b"], dtype=np.float32)

    WT = np.ascontiguousarray(W.T)
    b128 = np.ascontiguousarray(bias.reshape(NC_D, 128).T)
    in_maps = []
    for i in range(NCORES):
        sl = slice(i * NB, (i + 1) * NB)
        in_maps.append(
            {
                "xT": np.ascontiguousarray(x[sl].T),
                "WT": WT,
                "bias": b128,
                "enc": np.ascontiguousarray(enc[:, sl, :]),
            }
        )
    return in_maps


def run(inputs: dict, trace: bool = False, tmpdir: str | None = None):
    """Returns (full_output [B, 2E] f32, exec_time_ns or None)."""
    nc = _get_nc()
    in_maps = make_in_maps(inputs)
    res = run_bass_kernel_spmd(
        nc, in_maps, core_ids=list(range(NCORES)), trace=trace, tmpdir=tmpdir
    )
    out = np.concatenate([res.results[i]["out"] for i in range(NCORES)], axis=0)
    return out.astype(np.float32), res.exec_time_ns


def kernel(**inputs) -> np.ndarray:
    out, _ = run(inputs, trace=False)
    return out


# revision 31
# speedup vs baseline: 1.4140x; 1.3662x over previous
"""Trainium2 Bass kernel for the attention module:

    xp      = x @ W.T + b                      # [B, E]
    scores  = einsum('be,tbe->bt', xp, enc)    # [B, T]
    attn    = softmax(scores, axis=1)
    context = einsum('bt,tbe->be', attn, enc)  # [B, E]
    out     = concat([xp, context], axis=1)    # [B, 2E]

Shapes: T=2048, B=128, D_dec=512, E=512 (fp32).

Strategy (data-parallel over batch, 8 NeuronCores, no collectives):
  - Each core owns NB=16 batches: its encoder_states shard is
    [T, 16, E] = 64 MiB fp32, streamed from HBM exactly once in NT=16
    t-tiles of [128, 16, 512], CAST TO FP16 during the SWDGE DMA
    (HBM read stays fp32 -> memory roofline ~188us; SBUF side halves).
    The first NBUF DMAs are issued before anything else so the HBM
    stream starts at t=0; 5 rotating buffers give the stream ~3 tiles
    of lookahead.
  - fp16 on-chip enc buys: DVE 2x_1p mode for the big multiply, and
    1 cycle/row PE matmuls for the context accumulation (fp32 would be
    4 cycles/row = 2 passes).  Score accumulation stays fp32.
  - Per tile k (flash-style, deferred softmax combine):
      prod     = enc * xp            (ONE fp16 DVE tensor_tensor)
      S[t,b]   = sum_e prod          (batches 0-7: one batched DVE
                 tensor_reduce; 8-15: ScalarE activation accum)
      sT       = S^T                 (TensorE transpose, fp32)
      -m_k     = -rowmax(sT)         (VectorE reduce)
      pT, l_k  = exp(sT - m_k) + rowsum (ScalarE activation, pT fp16)
      p        = pT^T                (TensorE transpose, fp16)
      c_k      = per-batch sum_t p * enc via 16 masked fp16 matmuls
                 accumulating one [16, 512] fp32 PSUM tile (TensorE)
    Tails run TWO tiles behind heads (pipeline depth 3) so the
    cross-engine tail chain never throttles the DMA stream.
  - Final: exact softmax combine over the 16 tiles' (m_k, l_k, c_k),
    fp32, split between VectorE (tiles 0-7) and GpSimd (tiles 8-15)
    so the post-stream serial chain is short.

This toolchain's walrus accepts AT MOST ONE semaphore wait per TPB
compute instruction, and Tile pool slot reuse emits extra release
waits.  Hence: hot buffers are allocated once and alternated manually,
and cheap "observer" ops make each engine see a new producer before
the real consumer runs, keeping every instruction at <= 1 wait.
"""

import os
import sys

import numpy as np

if "/opt/trn_rl_repo" not in sys.path and not any(
    os.path.isdir(os.path.join(p, "concourse")) for p in sys.path if p
):
    sys.path.insert(0, "/opt/trn_rl_repo")

import concourse.bass as bass
import concourse.mybir as mybir
import concourse.tile as tile
from concourse.bass_utils import run_bass_kernel_spmd
from concourse.masks import make_identity
from concourse.tile_rust import add_dep_helper

T, B, D, E = 2048, 128, 512, 512
NCORES = 8
NB = B // NCORES  # 16 local batches per core
PT = 128          # t-tile partition size
NT = T // PT      # 16 t-tiles
NC_D = D // 128   # 4 chunks of the contraction dim for the xp matmul
NBUF = 6          # rotating fp16 enc tile buffers
NDVE = 8          # batches reduced by the batched DVE tensor_reduce
LAG = 2           # tail(k-LAG) is emitted after head(k)

F32 = mybir.dt.float32
F16 = mybir.dt.float16
AF = mybir.ActivationFunctionType
ALU = mybir.AluOpType
AX = mybir.AxisListType


def _install_drain_split():
    """This walrus rejects instructions carrying more than one semaphore
    wait.  Tile's kernel-tail drain waits on every proc's final tick in a
    single instruction; split it into one drain per wait."""
    from concourse.vector_clock import ScopedClock

    if getattr(tile.TileContext, "_drain_split_installed", False):
        return

    def _split_dab(self, tick_clock, wait_clock):
        drain_inst = self.nc.sync.drain()
        wait_clock.add_sem_waits(
            drain_inst.ins, ScopedClock({None: tick_clock.global_clock})
        )
        si = drain_inst.ins.sync_info
        if si is not None and len(si.on_wait) > 1:
            waits = list(si.on_wait)
            upds = list(si.on_update)
            drain_inst.ins.sync_info = mybir.SyncInfo(
                on_wait=[waits[0]], on_update=upds
            )
            for w in waits[1:]:
                d2 = self.nc.sync.drain()
                d2.ins.sync_info = mybir.SyncInfo(on_wait=[w], on_update=[])

        self.nc.all_engine_barrier()
        assert self.sems is not None
        popped = self.nc._tile_sem_poison_stack.pop()
        assert popped is self._sem_poison
        self.nc.clear_and_free_semaphores(list(self.sems.allocated().values()))
        self.nc.all_engine_barrier()

    tile.TileContext._drain_and_barrier = _split_dab
    tile.TileContext._drain_split_installed = True


_install_drain_split()


def build_nc() -> bass.Bass:
    nc = bass.Bass()

    # Per-core shards (host pre-transposes the small operands for layout).
    xT_ext = nc.declare_dram_parameter("xT", [D, NB], F32, isOutput=False)
    WT_ext = nc.declare_dram_parameter("WT", [D, E], F32, isOutput=False)
    b_ext = nc.declare_dram_parameter("bias", [128, NC_D], F32, isOutput=False)
    enc_ext = nc.declare_dram_parameter("enc", [T, NB, E], F32, isOutput=False)
    out_ext = nc.declare_dram_parameter("out", [NB, 2 * E], F32, isOutput=True)

    with tile.TileContext(nc) as tc:
        with (
            tc.tile_pool(name="sb", bufs=1) as sb,
            tc.tile_pool(name="dram", bufs=1, space="DRAM") as dram_pool,
            tc.tile_pool(name="ps", bufs=1, space="PSUM") as ps,
        ):
            # ---- DMA ordering note ---------------------------------------
            # The SDMA engines strongly favor the SWDGE (Pool) queue while
            # it has queued work: HWDGE transfers starve behind a deep
            # SWDGE backlog.  So EVERYTHING the critical path needs rides
            # the one SWDGE FIFO, in exactly the order it is needed:
            #   xT/WT/b -> enc(0) -> [xp bounce + broadcast] -> enc(1..)
            enc_t = [sb.tile([PT, NB, E], F16, name=f"enc{i}") for i in range(NBUF)]
            dmas = {}

            xT_sb = sb.tile([128, NC_D, NB], F32)
            nc.gpsimd.dma_start(
                out=xT_sb[:], in_=xT_ext[:, :].rearrange("(c p) b -> p c b", p=128)
            )
            WT_sb = sb.tile([128, NC_D, E], F32)
            nc.gpsimd.dma_start(
                out=WT_sb[:], in_=WT_ext[:, :].rearrange("(c p) e -> p c e", p=128)
            )
            b_sb = sb.tile([128, NC_D], F32)
            nc.gpsimd.dma_start(out=b_sb[:], in_=b_ext[:, :])

            dmas[0] = nc.gpsimd.dma_start(
                out=enc_t[0][:], in_=enc_ext[0:PT, :, :]
            )

            ident = sb.tile([128, 128], F32)
            make_identity(nc, ident[:])
            identH = sb.tile([128, 128], F16)
            make_identity(nc, identH[:])

            obs_ps1 = ps.tile([1, 16], F32, tag="obs1")
            obs_ps2 = ps.tile([1, 16], F32, tag="obs2")

            def pe_observe(ap, obs):
                return nc.tensor.matmul(obs[:], lhsT=ap[:, 0:1], rhs=ap[:, 0:16],
                                        start=True, stop=True)

            pe_observe(ident, obs_ps1)   # PE observes the fp32 identity
            pe_observe(identH, obs_ps1)  # PE observes the fp16 identity

            # ---- setup: xp = x @ W.T + b ----------------------------------
            obs_xt = pe_observe(xT_sb[:, 0, :], obs_ps2)  # PE observes xT DMA
            junk_b = sb.tile([128, 1], F32)
            nc.vector.tensor_copy(junk_b[:], b_sb[:, 0:1])  # DVE observes b DMA

            # xp^T chunks: [128 (e-local), ce, b]
            xpT_sb = sb.tile([128, NC_D, NB], F32)
            ps_xpT = ps.tile([128, NB], F32, tag="ps_xpT")
            for ce in range(NC_D):
                for cd in range(NC_D):
                    mm = nc.tensor.matmul(
                        ps_xpT[:],
                        lhsT=WT_sb[:, cd, ce * 128 : (ce + 1) * 128],
                        rhs=xT_sb[:, cd, :],
                        start=(cd == 0),
                        stop=(cd == NC_D - 1),
                    )
                    if ce == 0 and cd == 0:
                        add_dep_helper(mm.ins, obs_xt.ins, sync=False)
                nc.vector.tensor_scalar_add(
                    xpT_sb[:, ce, :], ps_xpT[:], b_sb[:, ce : ce + 1]
                )

            # out[:, 0:E] = xp (natural layout) via TensorE transposes
            out_tile = sb.tile([NB, 2 * E], F32)
            ps_xp = ps.tile([NB, 128], F32, tag="ps_xp")
            for ce in range(NC_D):
                nc.tensor.transpose(ps_xp[:], xpT_sb[:, ce, :], ident[:])
                # DVE (not ACT) so out_tile has a single producer engine
                nc.vector.tensor_copy(
                    out_tile[:, ce * 128 : (ce + 1) * 128], ps_xp[:]
                )

            # The xp half of the output never changes after this point; ship
            # it now so the final output DMA only covers the context half.
            # (HWDGE: not latency-critical, must only land by kernel end.)
            nc.sync.dma_start(out=out_ext[:, 0:E], in_=out_tile[:, 0:E])

            # Broadcast xp (as fp16) to all 128 partitions via DRAM bounce
            # with a 0-stride partition dim on the read side.  Both legs
            # ride the SWDGE FIFO right behind enc(0), ahead of enc(1..),
            # so the broadcast lands ~20us in instead of starving behind
            # the whole enc backlog.
            xp16 = sb.tile([NB, E], F16)
            nc.vector.tensor_copy(xp16[:], out_tile[:, 0:E])
            xp_dram = dram_pool.tile([NB, E], F16)
            nc.gpsimd.dma_start(out=xp_dram[:], in_=xp16[:])
            xpb = sb.tile([128, NB, E], F16)
            nc.gpsimd.dma_start(out=xpb[:], in_=xp_dram[:].partition_broadcast(128))

            # Now flood the rest of the initial enc window.
            for k in range(1, NBUF):
                dmas[k] = nc.gpsimd.dma_start(
                    out=enc_t[k][:], in_=enc_ext[k * PT : (k + 1) * PT, :, :]
                )

            # ---- persistent buffers for the t-tile loop -------------------
            # Write-once column layouts (one column/slice per t-tile) avoid
            # same-engine WAW hazards entirely; prod/pm/ctx alternate
            # manually.
            prod = [sb.tile([PT, NB, E], F16, name=f"prod{i}") for i in range(2)]
            S_all = sb.tile([PT, NT, NB], F32)
            pT_all = sb.tile([NB, NT, PT], F16)
            # 4 rotating mask buffers: the diag-write WAW distance is then
            # long enough that its same-engine posted-write wait always
            # elides, so the diag copy carries only the PE wait.
            pm_t = [sb.tile([PT, NB, NB], F16, name=f"pm{i}") for i in range(4)]
            for t in pm_t:
                nc.vector.memset(t[:], 0.0)  # off-diagonals stay 0 forever
            junk_es = sb.tile([PT, NT], F32)
            junk_ss = sb.tile([NB, NT], F32)
            dummy_all = sb.tile([PT, NT, NB], F32)  # write-once reduce dummies
            jw = sb.tile([1, 1], F32)
            jns = sb.tile([NB, NT], F32)

            sT_ps = ps.tile([NB, PT], F32, tag="sT")
            p_ps = ps.tile([PT, NB], F16, tag="p")
            ctx_pair = [ps.tile([NB, E], F32, name=f"ctx{i}") for i in range(2)]

            NEGM = sb.tile([NB, NT], F32)   # -m_k per (b, k)
            L_all = sb.tile([NB, NT], F32)  # l_k per (b, k)
            c_store = sb.tile([NB, NT, E], F16)

            # ---- software-pipelined t-tile loop --------------------------
            # head(k): enc DMA + multiply + reduces for tile k
            # tail(k-2): transposes/softmax/context matmuls for tile k-2,
            # emitted after head(k) so the cross-engine tail chain (which is
            # longer than one DMA period) is pipelined three tiles deep.
            hist = {}
            tts = {}
            cstore_q = []  # tiles whose ctx_ps copy is still pending

            def emit_tail(k):
                eth = enc_t[k % NBUF]
                ctx_ps = ctx_pair[k % 2]
                st_inst = nc.tensor.transpose(sT_ps[:], S_all[:, k, :], ident[0:PT, 0:PT])
                if cstore_q:
                    # the pending c_store copy must land before this tile's
                    # ctx matmuls reuse ctx_ps; syncing the S transpose on it
                    # both orders the streams and keeps jmm at one wait
                    add_dep_helper(st_inst.ins, cstore_q[-1].ins, sync=True)
                if k >= 1:
                    # DVE observes exp(k-1) (the last sT_ps reader) so the
                    # NEGM reduce carries only the PE wait
                    nc.vector.tensor_copy(jns[:, k : k + 1], pT_all[:, k - 1, 0:1])
                nc.vector.tensor_reduce(
                    out=NEGM[:, k : k + 1], in_=sT_ps[:], axis=AX.X, op=ALU.max,
                    negate=True,
                )
                nc.scalar.activation(junk_ss[:, k : k + 1], sT_ps[:, 0:1], AF.Copy)
                nc.scalar.activation(
                    pT_all[:, k, :], sT_ps[:], AF.Exp,
                    bias=NEGM[:, k : k + 1], scale=1.0,
                    accum_out=L_all[:, k : k + 1],
                )
                nc.tensor.transpose(p_ps[:], pT_all[:, k, :], identH[0:NB, 0:NB])
                pm = pm_t[k % 4]
                nc.vector.tensor_copy(
                    pm[:, :, :].rearrange("p a b -> p (a b)")[:, :: NB + 1],
                    p_ps[:],
                )
                # PE observes the enc DMA before the ctx matmuls use it
                jmm = nc.tensor.matmul(
                    ctx_ps[0:1, 0:16],
                    lhsT=eth[:, 0, 0:1], rhs=eth[:, 0, 0:16],
                    start=True, stop=True,
                )
                add_dep_helper(jmm.ins, st_inst.ins, sync=False)
                last_mm = None
                for b in range(NB):
                    last_mm = nc.tensor.matmul(
                        ctx_ps[:],
                        lhsT=pm[:, b, :],
                        rhs=eth[:, b, :],
                        start=(b == 0),
                        stop=(b == NB - 1),
                    )
                hist[k % NBUF] = (last_mm, tts[k][0], tts[k][1], dmas[k])
                return last_mm

            def emit_cstore(k):
                inst = nc.scalar.activation(
                    c_store[:, k, :], ctx_pair[k % 2][:], AF.Copy
                )
                cstore_q.append(inst)
                return inst

            for k in range(NT):
                eth = enc_t[k % NBUF]
                pr = prod[k % 2]
                # Pre-absorb the buffer's reuse hazards on the Pool proc
                # with explicitly-synced nops, so the SWDGE DMA needs no
                # more than the allowed number of waits.
                if k >= NBUF:
                    for dep in hist[k % NBUF]:
                        nop = nc.gpsimd.engine_nop()
                        add_dep_helper(nop.ins, dep.ins, sync=True)
                    dmas[k] = nc.gpsimd.dma_start(
                        out=eth[:], in_=enc_ext[k * PT : (k + 1) * PT, :, :]
                    )
                # For early tiles the jns elision chain doesn't exist yet, so
                # a DVE observer carries the enc DMA wait; from k=4 on the
                # multiply itself carries the DMA semaphore (its prod WAR
                # elides via the jns chain).
                if k < 4:
                    nc.vector.tensor_copy(junk_es[:, k : k + 1], eth[:, 0, 0:1])

                # prod = enc * xp, split in two halves aligned with the
                # reduce split: half a feeds the batched DVE reduce (WAR
                # from tile k-2 is DVE-only), half b feeds the ScalarE
                # accumulates, which start as soon as half b's write ack
                # returns — overlapping the DVE reduce.
                nd = NB if k == 0 else NDVE
                tt_a = nc.vector.tensor_tensor(
                    out=pr[:, 0:NDVE, :], in0=eth[:, 0:NDVE, :],
                    in1=xpb[:, 0:NDVE, :], op=ALU.mult,
                )
                tt_b = nc.vector.tensor_tensor(
                    out=pr[:, NDVE:NB, :], in0=eth[:, NDVE:NB, :],
                    in1=xpb[:, NDVE:NB, :], op=ALU.mult,
                )
                tts[k] = (tt_a, tt_b)

                # Scores: batched DVE reduce for the first NDVE batches
                # (tile 0: all 16, so S_all[:,0,:] has a single producer
                # engine and the first tail's transpose carries one wait),
                # ScalarE activation-accumulate for the rest.
                nc.vector.tensor_reduce(
                    out=S_all[:, k, 0:nd], in_=pr[:, 0:nd, :], axis=AX.X,
                    op=ALU.add,
                )
                for i, b in enumerate(range(nd, NB)):
                    nc.scalar.activation(
                        dummy_all[:, k, b : b + 1].broadcast_to((PT, E)),
                        pr[:, b, :],
                        AF.Copy,
                        accum_out=S_all[:, k, b : b + 1],
                    )
                    if i == 0 and k >= LAG + 1:
                        emit_cstore(k - LAG - 1)

                if k >= LAG:
                    emit_tail(k - LAG)

            emit_tail(NT - 2)
            emit_cstore(NT - 3)
            emit_tail(NT - 1)
            emit_cstore(NT - 2)
            emit_cstore(NT - 1)
            prev_cstore = cstore_q[-1]

            # ---- final combine across tiles -------------------------------
            negM = sb.tile([NB, 1], F32)
            nc.vector.tensor_reduce(out=negM[:], in_=NEGM[:], axis=AX.X, op=ALU.min)
            alpha = sb.tile([NB, NT], F32)
            # alpha = exp(-NEGM * 1 + (-M)) = exp(m_k - M); ordered after the
            # last c_store copy so the combine loop's ACT waits all elide
            ainst = nc.scalar.activation(
                alpha[:], NEGM[:], AF.Exp, bias=negM[:], scale=-1.0
            )
            add_dep_helper(ainst.ins, prev_cstore.ins, sync=False)
            prodw = sb.tile([NB, NT], F32)
            nc.vector.tensor_tensor(out=prodw[:], in0=alpha[:], in1=L_all[:],
                                    op=ALU.mult)
            Lsum = sb.tile([NB, 1], F32)
            nc.vector.tensor_reduce(out=Lsum[:], in_=prodw[:], axis=AX.X, op=ALU.add)
            rL = sb.tile([NB, 1], F32)
            nc.vector.reciprocal(rL[:], Lsum[:])
            w = sb.tile([NB, NT], F32)
            nc.vector.tensor_scalar_mul(w[:], alpha[:], rL[:])

            # Weighted sum of the 16 stored contexts on DVE as TWO
            # interleaved accumulation chains, so each op's posted-write
            # self-wait hides under the other chain's execution.  All ACT
            # (c_store) waits elide because prodw already waited on alpha,
            # which is ordered after the last c_store copy.
            acc = out_tile[:, E : 2 * E]
            acc_b = sb.tile([NB, E], F32)
            nc.vector.tensor_copy(jw[:], w[0:1, 0:1])  # absorb w's self-wait
            nc.vector.tensor_scalar_mul(acc, c_store[:, 0, :], w[:, 0:1])
            h = NT // 2
            nc.vector.tensor_scalar_mul(acc_b[:], c_store[:, h, :], w[:, h : h + 1])
            for k in range(1, h):
                nc.vector.scalar_tensor_tensor(
                    out=acc, in0=c_store[:, k, :], scalar=w[:, k : k + 1], in1=acc,
                    op0=ALU.mult, op1=ALU.add,
                )
                nc.vector.scalar_tensor_tensor(
                    out=acc_b[:], in0=c_store[:, h + k, :],
                    scalar=w[:, h + k : h + k + 1], in1=acc_b[:],
                    op0=ALU.mult, op1=ALU.add,
                )
            nc.vector.tensor_tensor(out=acc, in0=acc, in1=acc_b[:], op=ALU.add)

            nc.sync.dma_start(out=out_ext[:, E : 2 * E], in_=acc)

    return nc


_NC_CACHE: bass.Bass | None = None


def _get_nc() -> bass.Bass:
    global _NC_CACHE
    if _NC_CACHE is None:
        _NC_CACHE = build_nc()
    return _NC_CACHE


def make_in_maps(inputs: dict) -> list[dict]:
    x = np.ascontiguousarray(np.asarray(inputs["x"], dtype=np.float32))
    enc = np.asarray(inputs["encoder_states"], dtype=np.float32)
    W = np.asarray(inputs["W"], dtype=np.float32)
    bias = np.asarray(inputs["b"], dtype=np.float32)

    WT = np.ascontiguousarray(W.T)
    b128 = np.ascontiguousarray(bias.reshape(NC_D, 128).T)
    in_maps = []
    for i in range(NCORES):
        sl = slice(i * NB, (i + 1) * NB)
        in_maps.append(
            {
                "xT": np.ascontiguousarray(x[sl].T),
                "WT": WT,
                "bias": b128,
                "enc": np.ascontiguousarray(enc[:, sl, :]),
            }
        )
    return in_maps


def run(inputs: dict, trace: bool = False, tmpdir: str | None = None):
    """Returns (full_output [B, 2E] f32, exec_time_ns or None)."""
    nc = _get_nc()
    in_maps = make_in_maps(inputs)
    res = run_bass_kernel_spmd(
        nc, in_maps, core_ids=list(range(NCORES)), trace=trace, tmpdir=tmpdir
    )
    out = np.concatenate([res.results[i]["out"] for i in range(NCORES)], axis=0)
    return out.astype(np.float32), res.exec_time_ns


def kernel(**inputs) -> np.ndarray:
    out, _ = run(inputs, trace=False)
    return out
